# revision 30
# baseline (speedup 1.0000x reference)
"""Trainium2 Bass kernel for CausalSelfAttention (PentaNet-quantized weights).

Reference computation (B=2, T=2048, C=1024, H=16 heads, D=64):
    qkv = x @ quant(w_attn).T ; split q,k,v ; causal softmax attention ;
    out = y @ quant(w_proj).T

Sharding: 8 cores = 2 (batch) x 4 (head groups of 4 heads).  Each core
computes its batch element's attention for its 4 heads plus the partial
output projection over its 256 input channels; the host sums the 4
partials per batch (the w_proj contraction is split across head groups).

Device layout avoids all on-chip transposes:
  - host supplies xT = x[b].T  [C, T]
  - qT,kT computed as [o, t] (weights stationary), v as [t, o]
  - scores computed transposed: ST[j, i] = k_j . q_i  (j = key pos)
  - P = exp(ST/8) with causal masking (per-ktile triangular mask)
  - OT[d, i] = sum_j V[j, d] P[j, i] accumulated in PSUM; an extra
    ones-column in V yields the softmax denominator as OT row 64
  - OT normalized is exactly the lhsT the projection needs.

Head-pair concurrency: heads (0,1) and (2,3) sit on SBUF partition rows
0-63 / 64-127 of the same q/k column blocks, so their QK^T matmuls are
64-contraction row-tiles at tile_position (0,0) / (64,0).  Emitting the
two matmuls back-to-back lets the PE run them concurrently (disjoint
row-groups), doubling effective QK throughput.  One exp covers both
heads' score tiles ([A | B] halves of one PSUM pair tile).

Projection results go straight from PSUM to DRAM via DMA (no SBUF
staging), freeing ACT/DVE cycles.  All matmuls run in bf16 (fp32 PSUM
accumulation) for full-rate streaming with pipelined fast-weight-load.
"""

import os
import sys

sys.path.insert(0, "/opt/trn_rl_repo")

import numpy as np
import ml_dtypes

import jax

try:
    jax.config.update("jax_compilation_cache_dir", "/root/.cache/jax_bass_neff")
except Exception:
    pass

import concourse.bass as bass
import concourse.tile as tile
from concourse import bacc, mybir
from concourse.bass_utils import run_bass_kernel_spmd

F32 = mybir.dt.float32
F32R = mybir.dt.float32r
BF16 = mybir.dt.bfloat16

B, T, C = 2, 2048, 1024
H, D = 16, 64
HL = 4                    # heads per core
OL = HL * D               # 256 local output channels
KT = C // 128             # 8 k-tiles over C
TT = T // 128             # 16 t-tiles
NCH = T // 512            # 4 i-chunks of 512
SCALE = 1.0 / 8.0         # 1/sqrt(D)


def r(ap):
    return ap


def build_body(ctx, tc, xT, wq, wk, wv, wp, tri, out):
    nc = tc.nc

    consts = ctx.enter_context(tc.tile_pool(name="consts", bufs=1))
    acts = ctx.enter_context(tc.tile_pool(name="acts", bufs=1))
    pp = ctx.enter_context(tc.tile_pool(name="pp", bufs=3))
    rcp = ctx.enter_context(tc.tile_pool(name="rcp", bufs=2))
    bbp = ctx.enter_context(tc.tile_pool(name="bbp", bufs=2))
    obp = ctx.enter_context(tc.tile_pool(name="obp", bufs=3))
    obh = ctx.enter_context(tc.tile_pool(name="obh", bufs=8))
    ps_mm = ctx.enter_context(tc.tile_pool(name="ps_mm", bufs=2, space="PSUM"))
    ps_pj = ctx.enter_context(tc.tile_pool(name="ps_pj", bufs=2, space="PSUM"))
    ps_ot = ctx.enter_context(tc.tile_pool(name="ps_ot", bufs=2, space="PSUM"))

    # ---- load inputs to SBUF ----
    # wq + xT chunk 0 first so the first qkv matmuls start ASAP
    wq_sb = consts.tile([128, KT * OL], BF16)
    wk_sb = consts.tile([128, KT * OL], BF16)
    wv_sb = consts.tile([128, KT * OL], BF16)
    xT_sb = consts.tile([128, KT * T], BF16)

    def load_w(w_sb, w_d):
        # one DMA: DRAM [KT*128, OL] -> SBUF [128, KT, OL]
        nc.sync.dma_start(
            w_sb[:].rearrange("p (k o) -> p k o", k=KT),
            w_d.rearrange("(k p) o -> p k o", k=KT))

    def load_x(n, k0=0, k1=KT):
        # one DMA: DRAM [k1-k0 x 128 rows, 512 cols] -> SBUF [128, k, 512]
        nc.sync.dma_start(
            xT_sb[:].rearrange("p (k t) -> p k t", k=KT)
            [:, k0:k1, n * 512:(n + 1) * 512],
            xT.rearrange("(k p) t -> p k t", k=KT)
            [:, k0:k1, n * 512:(n + 1) * 512])

    def load_w_half(w_sb, w_d, k0, k1):
        nc.sync.dma_start(
            w_sb[:].rearrange("p (k o) -> p k o", k=KT)[:, k0:k1],
            w_d.rearrange("(k p) o -> p k o", k=KT)[:, k0:k1])

    load_w_half(wq_sb, wq, 0, KT // 2)
    load_x(0, 0, 1)
    load_w_half(wq_sb, wq, KT // 2, KT)
    for k in range(1, KT):
        load_x(0, k, k + 1)
    load_w(wk_sb, wk)
    load_w(wv_sb, wv)
    tri_sb = consts.tile([128, 128], BF16)
    nc.sync.dma_start(tri_sb[:], tri[:, 0:128])
    for n in range(1, NCH):
        load_x(n)
    wp_sb = consts.tile([128, 2 * C], BF16)
    nc.sync.dma_start(
        wp_sb[:].rearrange("p (k o) -> p k o", k=2),
        wp.rearrange("(k p) o -> p k o", k=2))

    q_sb = acts.tile([128, 2 * T], BF16)
    k_sb = acts.tile([128, 2 * T], BF16)
    v_sb = acts.tile([128, TT * HL * (D + 1)], BF16)
    yt_sb = acts.tile([128, 2 * T], BF16)
    # ones column (index D) of every [t-tile, head] V block, built on ACT
    # (a strided DMA here would be descriptor-bound): 1.0 = tri*0 + 1
    v_ones = v_sb[:].rearrange("p (g c) -> p g c", c=D + 1)[:, :, D]
    nc.scalar.activation(v_ones, tri_sb[:, 0:TT * HL],
                         mybir.ActivationFunctionType.Copy, scale=0.0, bias=1.0)

    # ---- emission units ----
    def qk_unit(n, which, m):
        w_sb, dst = (wq_sb, q_sb) if which == 0 else (wk_sb, k_sb)
        ps = ps_pj.tile([128, 512], F32, tag="pj")
        for k in range(KT):
            nc.tensor.matmul(
                ps[:],
                r(w_sb[:, k * OL + m * 128: k * OL + (m + 1) * 128]),
                r(xT_sb[:, k * T + n * 512: k * T + (n + 1) * 512]),
                start=(k == 0), stop=(k == KT - 1),
            )
        nc.vector.tensor_copy(dst[:, m * T + n * 512: m * T + (n + 1) * 512], ps[:])

    def v_unit(t):
        ps = ps_pj.tile([128, OL], F32, tag="pj")
        for k in range(KT):
            nc.tensor.matmul(
                ps[:],
                r(xT_sb[:, k * T + t * 128: k * T + (t + 1) * 128]),
                r(wv_sb[:, k * OL:(k + 1) * OL]),
                start=(k == 0), stop=(k == KT - 1),
            )
        dst = v_sb[:, t * HL * (D + 1): (t + 1) * HL * (D + 1)]
        dst = dst.rearrange("p (h c) -> p h c", h=HL)[:, :, 0:D]
        nc.vector.tensor_copy(dst, ps[:].rearrange("p (h c) -> p h c", h=HL))

    def proj_unit(t, n2, dve_only=True, split_store=False, kk_order=(0, 1)):
        # n2 == 0 computes cols [0:512] into ob; n2 == 1 computes [512:1024]
        # and issues the store(s) for the whole t-tile row block.
        ps = ps_pj.tile([128, 512], F32, tag="pj")
        for i, kk in enumerate(kk_order):
            nc.tensor.matmul(
                ps[:],
                r(yt_sb[:, kk * T + t * 128: t * 128 + kk * T + 128]),
                r(wp_sb[:, kk * C + n2 * 512: kk * C + (n2 + 1) * 512]),
                start=(i == 0), stop=(i == 1),
            )
        ob = _ob_state.get(t)
        if ob is None:
            ob = obp.tile([128, 1024], BF16, tag="ob", name=f"ob_{t}")
            _ob_state[t] = ob
        if n2 == 0:
            nc.vector.tensor_copy(ob[:, 0:512], ps[:])
            if split_store:
                nc.sync.dma_start(out[t * 128:(t + 1) * 128, 0:512], ob[:, 0:512])
        else:
            if dve_only:
                nc.vector.tensor_copy(ob[:, 512:1024], ps[:])
            else:
                nc.scalar.copy(ob[:, 512:1024], ps[:])
            if split_store:
                nc.sync.dma_start(out[t * 128:(t + 1) * 128, 512:1024], ob[:, 512:1024])
            else:
                nc.sync.dma_start(out[t * 128:(t + 1) * 128, :], ob[:])
            del _ob_state[t]

    _ob_state = {}

    def qkv_units(n):
        return ([(lambda n=n, w=w, m=m: qk_unit(n, w, m)) for w in range(2) for m in range(2)]
                + [(lambda t=t: v_unit(t)) for t in range(4 * n, 4 * n + 4)])

    def proj_units(ic, dve_only=True, split_store=False, kk_order=(0, 1)):
        return [(lambda t=t, n2=n2: proj_unit(t, n2, dve_only, split_store, kk_order))
                for t in range(4 * ic, 4 * ic + 4) for n2 in range(2)]

    # last-chunk projection, split by kk half: the kk=1 matmul (heads 2,3 --
    # the pair processed FIRST in the last chunk) runs as soon as that pair's
    # epilogue lands and stages into SBUF; the trailing kk=0 half is added
    # in on DVE and stored.
    _pj3_state = {}

    def pj3_half(t, n2, kk):
        ps = ps_pj.tile([128, 512], F32, tag="pj")
        nc.tensor.matmul(
            ps[:],
            r(yt_sb[:, kk * T + t * 128: t * 128 + kk * T + 128]),
            r(wp_sb[:, kk * C + n2 * 512: kk * C + (n2 + 1) * 512]),
            start=True, stop=True,
        )
        if kk == 1:
            ob = obh.tile([128, 512], BF16, tag="obh", name=f"obh_{t}_{n2}")
            _pj3_state[t, n2] = ob
            nc.vector.tensor_copy(ob[:], ps[:])
        else:
            ob = _pj3_state.pop((t, n2))
            nc.vector.tensor_add(ob[:], ps[:], ob[:])
            nc.sync.dma_start(out[t * 128:(t + 1) * 128, n2 * 512:(n2 + 1) * 512],
                              ob[:])

    def pj3_units(kk):
        return [(lambda t=t, n2=n2: pj3_half(t, n2, kk))
                for t in range(4 * (NCH - 1), 4 * (NCH - 1) + 4) for n2 in range(2)]

    # ---- attention chunk: head pairs (0,1) / (2,3), ktile-major ----
    # Pair hp: head A = 2hp (partitions 0-63), head B = 2hp+1 (64-127),
    # q/k column block mo = hp*T.  QK^T for A and B are emitted
    # back-to-back (concurrent row-tiles on HW); one exp covers both.
    def attn_blocks(ic):
        items = []
        nt = 4 * ic + 4
        # last chunk runs pair (2,3) first so the trailing projection's
        # kk=1 matmuls (heads 2,3) are ready during pair (0,1)'s epilogue
        hps = (1, 0) if ic == NCH - 1 else (0, 1)
        for hp in hps:
            mo = hp * T
            state = {}

            def open_ps(hp=hp, ic=ic, state=state):
                for hh in range(2):
                    state[hh] = ps_ot.tile([D + 1, 512], F32, tag="ot",
                                           name=f"ps_o_{ic}_{hp}_{hh}")

            def qk_block(tj, hp=hp, mo=mo, ic=ic, state=state):
                # diag tile: d = tj - 4*ic >= 0 -> cols cs..512, width w
                d = tj - 4 * ic
                cs = d * 128 if d >= 0 else 0
                w = 512 - cs
                ps_s = ps_mm.tile([128, 1024], F32, tag="mm", name=f"ps_s_{ic}_{hp}")
                for hh in range(2):
                    pb = 64 * hh
                    # B half always at col 512 (PSUM-bank aligned)
                    nc.tensor.matmul(
                        ps_s[:, hh * 512: hh * 512 + w],
                        r(k_sb[pb:pb + 64, mo + tj * 128: mo + (tj + 1) * 128]),
                        r(q_sb[pb:pb + 64, mo + ic * 512 + cs: mo + (ic + 1) * 512]),
                        start=True, stop=True,
                        skip_group_check=True,
                    )
                p_t = pp.tile([128, 1024], BF16, tag="p", name=f"p_t_{ic}_{hp}")
                src = ps_s[:].rearrange("p (h q) -> p h q", h=2)[:, :, 0:w]
                dst = p_t[:, 0:2 * w].rearrange("p (h w) -> p h w", h=2)
                nc.scalar.activation(dst, src,
                                     mybir.ActivationFunctionType.Exp, scale=SCALE)
                if d >= 0:
                    # mask the diagonal 128-col block of each half
                    nc.vector.tensor_mul(p_t[:, 0:128], p_t[:, 0:128], tri_sb[:])
                    nc.vector.tensor_mul(p_t[:, w:w + 128], p_t[:, w:w + 128], tri_sb[:])
                state["p", tj] = (p_t, w)

            def pv_block(tj, hp=hp, ic=ic, state=state, open_ps=open_ps):
                if tj == 0:
                    open_ps()
                p_t, w = state.pop(("p", tj))
                cs = 512 - w
                for hh in range(2):
                    h = 2 * hp + hh
                    vh = v_sb[:, (tj * HL + h) * (D + 1):(tj * HL + h + 1) * (D + 1)]
                    nc.tensor.matmul(
                        state[hh][:, cs:512],
                        r(vh), r(p_t[:, hh * w: hh * w + w]),
                        start=(tj == 0), stop=(tj == nt - 1),
                        skip_group_check=True,
                    )

            def epilogue(hh, hp=hp, mo=mo, ic=ic, state=state):
                # yt[o, i] = OT[d, i] / OT[D, i].  Copy the unnormalized OT
                # out first so the ps_ot bank frees early (the next pair's
                # PV accumulation reuses it); normalize in place after.
                # The very last pair splits the mul per t-tile so trailing
                # projection matmuls unblock sooner.
                ps_o = state[hh]
                pb = 64 * hh
                yt_sl = yt_sb[pb:pb + 64, mo + ic * 512: mo + (ic + 1) * 512]
                rc = rcp.tile([1, 512], F32R, tag="rc")
                with nc.allow_low_precision(reason="fp32r ~ fp32 denom"):
                    nc.vector.reciprocal(rc[:], ps_o[D:D + 1, :])
                if ic == NCH - 1 and hp == hps[-1]:
                    # ACT is exp-idle here; keep DVE free for the recips/muls
                    nc.scalar.copy(yt_sl, ps_o[0:D, :])
                else:
                    nc.vector.tensor_copy(yt_sl, ps_o[0:D, :])
                # bb spans all 128 partitions so the slice used below has
                # the same base partition as yt_sl (walrus SBUF constraint)
                bb = bbp.tile([128, 512], F32R, tag="bb")
                nc.gpsimd.partition_broadcast(bb[:], rc[:])
                bh = bb[pb:pb + 64, :]
                if ic == NCH - 1 and hp == hps[-1]:
                    for q4 in range(4):
                        nc.vector.tensor_mul(yt_sl[:, q4 * 128:(q4 + 1) * 128],
                                             yt_sl[:, q4 * 128:(q4 + 1) * 128],
                                             bh[:, q4 * 128:(q4 + 1) * 128])
                else:
                    nc.vector.tensor_mul(yt_sl, yt_sl, bh)

            for tj in range(nt):
                items.append(lambda tj=tj, f=qk_block: f(tj))
                if tj > 0:
                    items.append(lambda tj=tj, f=pv_block: f(tj - 1))
            items.append(lambda f=pv_block: f(nt - 1))
            items.append(lambda f=epilogue: f(0))
            items.append(lambda f=epilogue: f(1))
        return items

    def emit_interleaved(blocks, fillers, tail_reserve=0):
        """Emit attention blocks with filler units spread evenly between.
        The last `tail_reserve` fillers are held back to cover the final
        epilogue chain's latency."""
        nf = len(fillers) - tail_reserve
        nb = len(blocks)
        fi = 0
        for i, blk in enumerate(blocks):
            blk()
            want = (i + 1) * nf // nb
            while fi < want:
                fillers[fi]()
                fi += 1
        while fi < len(fillers):
            fillers[fi]()
            fi += 1

    # schedule: qkv(0) first; attention chunk ic interleaves qkv(ic+1);
    # all proj chunks 0-2 fill the ACT-bound chunk 3 (ob copies on DVE
    # there -- ACT is exp-saturated); proj(3) trails with split stores.
    for u in qkv_units(0):
        u()
    for ic in range(NCH):
        fill = []
        if ic + 1 < NCH:
            fill += qkv_units(ic + 1)
        if ic == NCH - 1:
            for pc in range(NCH - 1):
                fill += proj_units(pc)
            fill += pj3_units(kk=1)
        emit_interleaved(attn_blocks(ic), fill)
    for u in pj3_units(kk=0):
        u()


def build_program(reps=1):
    from contextlib import ExitStack

    nc = bacc.Bacc("TRN2", target_bir_lowering=False, debug=False)
    xT = nc.dram_tensor("xT", [C, T], BF16, kind="ExternalInput").ap()
    wq = nc.dram_tensor("wq", [C, OL], BF16, kind="ExternalInput").ap()
    wk = nc.dram_tensor("wk", [C, OL], BF16, kind="ExternalInput").ap()
    wv = nc.dram_tensor("wv", [C, OL], BF16, kind="ExternalInput").ap()
    wp = nc.dram_tensor("wp", [OL, C], BF16, kind="ExternalInput").ap()
    tri = nc.dram_tensor("tri", [128, 128], BF16, kind="ExternalInput").ap()
    out = nc.dram_tensor("out", [T, C], BF16, kind="ExternalOutput").ap()

    with tile.TileContext(nc) as tc:
        for _ in range(reps):
            with ExitStack() as ctx:
                build_body(ctx, tc, xT, wq, wk, wv, wp, tri, out)
    nc.compile()
    return nc


def quant_weight_np(w):
    scale = max(np.mean(np.abs(w), dtype=np.float32), np.float32(1e-8))
    return (np.clip(np.round(w / scale), -2.0, 2.0) * scale).astype(np.float32)


def make_in_maps(x, w_attn, w_proj):
    wq_f = quant_weight_np(w_attn)
    wp_f = quant_weight_np(w_proj)
    tri = np.triu(np.ones((128, 128), dtype=np.float32))
    in_maps = []
    for core in range(8):
        b, g = divmod(core, 4)
        sl = slice(g * OL, (g + 1) * OL)
        in_maps.append({
            "xT": np.ascontiguousarray(x[b].T).astype(ml_dtypes.bfloat16),
            "wq": np.ascontiguousarray(wq_f[0 * C:1 * C][sl].T).astype(ml_dtypes.bfloat16),
            "wk": np.ascontiguousarray(wq_f[1 * C:2 * C][sl].T).astype(ml_dtypes.bfloat16),
            "wv": np.ascontiguousarray(wq_f[2 * C:3 * C][sl].T).astype(ml_dtypes.bfloat16),
            "wp": np.ascontiguousarray(wp_f[:, sl].T).astype(ml_dtypes.bfloat16),
            "tri": tri.astype(ml_dtypes.bfloat16),
        })
    return in_maps


_CACHED_NC = None


def kernel(x, w_attn, w_proj):
    global _CACHED_NC
    if _CACHED_NC is None:
        _CACHED_NC = build_program()
    in_maps = make_in_maps(np.asarray(x, dtype=np.float32),
                           np.asarray(w_attn, dtype=np.float32),
                           np.asarray(w_proj, dtype=np.float32))
    res = run_bass_kernel_spmd(_CACHED_NC, in_maps, list(range(8)))
    out = np.zeros((B, T, C), dtype=np.float32)
    for core in range(8):
        b = core // 4
        out[b] += res.results[core]["out"].astype(np.float32)
    return out
